# revision 1
# baseline (speedup 1.0000x reference)
"""LocalAttention (B=1, S=4096, D=1024, H=16, hd=64, window=128) on 8 trn2 cores.

Sharding: sequence-parallel. Core c owns queries [512c, 512c+512) and receives
a key/value halo slice of 768 rows ([512c-128, 512c+640), zero-padded at the
global edges). All projection weights are replicated (bf16). Everything on
device runs in bf16 with fp32 PSUM accumulation.

Per-core dataflow:
  Fine-grained per-slab input DMAs ordered exactly as the Q projection
  consumes them (first matmul fires after ~0.4MB instead of ~6MB).
  qT = (Wq^T x^T) in [e, s] layout; kT likewise over the 768-col halo range;
  v = (Vin Wv) in natural [s, e] layout + a ones-column per head (softmax
  denominator rides along col 64 of each head's 65-wide v group).
  Scores are computed kb-major: for each (head, key-block) ONE matmul
  [64 x (128k, W q)] where W spans the 1..3 query blocks within the window,
  into a [128, W] PSUM tile; exp (scale=1/8, bf16, no max-subtract); then
  bf16 multiplicative masks on only the 1-2 diagonal-adjacent 128-col blocks
  (per-core host data bakes in the window triangles and the global-edge zero
  blocks). Each score tile is shared by up to 3 q-blocks via a rolling qb
  loop with V-projection chains interleaved into the attention stream.
  PV per (head, q-block): 3 accumulating matmuls lhsT=expp slice,
  rhs=[v_h | 1] -> [128, 65]; DVE reciprocal + tensor_scalar normalize into
  ao[qb] ([q, e] bf16). ao -> aoT via DMA XBAR transposes issued in halves
  (first half as soon as heads 0-7 are normalized) so the PE-side output
  projection (inline per qb, accumulating 8 e-blocks into [128,512] PSUM)
  never waits on the XBAR. Out staging copies are split across Vector and
  Scalar engines, DMA'd per 512-col chunk.
"""

import os

import numpy as np
import ml_dtypes

import concourse.bass as bass
import concourse.bacc as bacc
import concourse.mybir as mybir
import concourse.tile as tile
from concourse.bass_utils import run_bass_kernel_spmd

BF16 = mybir.dt.bfloat16
FP32 = mybir.dt.float32

NCORES = 8
S = 4096
D = 1024
H = 16
HD = 64
E = H * HD  # 1024
WIN = 128
SL = S // NCORES       # 512 queries per core
SK = SL + 2 * WIN      # 768 keys/values incl. halo
NQB = SL // 128        # 4 query blocks
NKB = SK // 128        # 6 key blocks
NDB = D // 128         # 8 contraction blocks
NEB = E // 128         # 8 embed blocks
VROW = HD + 1          # 65: v columns per head incl. ones column

# kb-major score tiles: valid q-blocks for key-block kb are
# [max(0, kb-2), min(NQB-1, kb)] (window = +-1 block around diagonal).
KB_Q0 = [max(0, kb - 2) for kb in range(NKB)]
KB_QN = [min(NQB - 1, kb) - max(0, kb - 2) + 1 for kb in range(NKB)]
KB_OFF = np.cumsum([0] + [n * 128 for n in KB_QN]).tolist()  # col offsets
SCORE_COLS = KB_OFF[-1]  # 1536

_CACHE = {}
LAST_RESULT = None  # BassKernelResults of the most recent run (for test.py)


def _build_nc():
    nc = bacc.Bacc("TRN2", target_bir_lowering=False, debug=False)

    qt_d = nc.dram_tensor("qt", [D, SL], BF16, kind="ExternalInput").ap()
    kt_d = nc.dram_tensor("kt", [D, SK], BF16, kind="ExternalInput").ap()
    vt_d = nc.dram_tensor("vt", [D, SK], BF16, kind="ExternalInput").ap()
    wq_d = nc.dram_tensor("wq", [D, E], BF16, kind="ExternalInput").ap()
    wk_d = nc.dram_tensor("wk", [D, E], BF16, kind="ExternalInput").ap()
    wv_d = nc.dram_tensor("wv", [D, E], BF16, kind="ExternalInput").ap()
    wo_d = nc.dram_tensor("wo", [E, D], BF16, kind="ExternalInput").ap()
    # bf16 multiplicative mask pairs (one strided DVE op covers blocks {0,2}
    # of a [128,384] score tile): [0:256]=[mk0|m0] for the kb0+kb1 pair,
    # [256:512]=[m2|m0] for kb2/kb3, [512:768]=[m2|mk5] for the kb4+kb5
    # pair. mk0/mk5 are m0/m2 with the global-edge zero blocks baked in.
    msk_d = nc.dram_tensor("msk", [128, 768], BF16, kind="ExternalInput").ap()
    out_d = nc.dram_tensor("out", [SL, D], FP32, kind="ExternalOutput").ap()

    with tile.TileContext(nc) as tc:
        pools = []

        def pool(name, bufs, **kw):
            p = tc.tile_pool(name=name, bufs=bufs, **kw)
            pools.append(p)
            return p.__enter__()

        const = pool("const", 1)
        psum = pool("psum", 2, space="PSUM")       # projections + out proj
        pscore = pool("pscore", 3, space="PSUM")   # score tiles [128, <=384]
        ppv_pool = pool("ppv", 3, space="PSUM")    # PV tiles [128, 65]
        ep = pool("expp", 3)                       # per-head exp tiles
        aop = pool("ao", 2)                        # per-qb attn-out [q, e]
        aotp = pool("aot", 8)                      # per-qb transposed [e, q]
        op = pool("o", 2)                          # per-qb fp32 out staging
        rp = pool("recip", 8)

        # ---- persistent SBUF tensors ----
        wq_sb = const.tile([128, NDB * E], BF16, tag="wq")
        wk_sb = const.tile([128, NDB * E], BF16, tag="wk")
        wv_sb = const.tile([128, NDB * E], BF16, tag="wv")
        wo_sb = const.tile([128, NEB * D], BF16, tag="wo")
        qtin_sb = const.tile([128, NDB * SL], BF16, tag="qtin")
        ktin_sb = const.tile([128, NDB * SK], BF16, tag="ktin")
        vtin_sb = const.tile([128, NDB * SK], BF16, tag="vtin")
        qt_sb = const.tile([128, NEB * SL], BF16, tag="qt")    # [e,s] per e-blk
        kt_sb = const.tile([128, NEB * SK], BF16, tag="kt")
        v_sb = const.tile([128, NKB * H * VROW], BF16, tag="v")  # [s, h*65]
        msk_sb = const.tile([128, 768], BF16, tag="msk")

        sync = nc.sync

        # ---- input DMAs: per-slab, ordered as consumed; alternate the two
        # HWDGE engines (SP / Activation) so descriptor issue is parallel ----
        def load_slab(sb, dr, ncols, b):
            eng = sync if b % 2 == 0 else nc.scalar
            eng.dma_start(
                sb[:, b * ncols:(b + 1) * ncols],
                dr[b * 128:(b + 1) * 128],
            )

        # Q-proj data: qtin slab + just the eb0 column block of each wq slab
        # first (all that the first PE chain needs), then the wq remainders.
        for db in range(NDB):
            load_slab(qtin_sb, qt_d, SL, db)
            eng = sync if db % 2 == 0 else nc.scalar
            eng.dma_start(
                wq_sb[:, db * E: db * E + 128],
                wq_d[db * 128:(db + 1) * 128, 0:128],
            )
        for db in range(NDB):
            eng = sync if db % 2 == 0 else nc.scalar
            eng.dma_start(
                wq_sb[:, db * E + 128:(db + 1) * E],
                wq_d[db * 128:(db + 1) * 128, 128:E],
            )
        for db in range(NDB):
            load_slab(ktin_sb, kt_d, SK, db)
            load_slab(wk_sb, wk_d, E, db)
        sync.dma_start(msk_sb[:], msk_d[:])
        for db in range(NDB):
            load_slab(vtin_sb, vt_d, SK, db)
            load_slab(wv_sb, wv_d, E, db)
        for eb in range(NEB):
            load_slab(wo_sb, wo_d, D, eb)

        # ones columns of v_sb (col hd=64 of each head group)
        v3 = v_sb[:].rearrange("p (k h c) -> p k h c", k=NKB, h=H)
        nc.gpsimd.memset(v3[:, :, :, HD:VROW], 1.0)

        # ---- q projection: [e, s] = Wq[d,e].T @ QT[d,s] ----
        for eb in range(NEB):
            ps = psum.tile([128, 512], FP32, tag="ps")
            for db in range(NDB):
                nc.tensor.matmul(
                    ps[:],
                    lhsT=wq_sb[:, db * E + eb * 128: db * E + (eb + 1) * 128],
                    rhs=qtin_sb[:, db * SL: db * SL + SL],
                    start=(db == 0),
                    stop=(db == NDB - 1),
                )
            nc.vector.tensor_copy(qt_sb[:, eb * SL:(eb + 1) * SL], ps[:])

        # ---- k projection: [e, s] = Wk[d,e].T @ KT[d,s] over halo range ----
        for eb in range(NEB):
            for s0, s1 in ((0, 512), (512, SK)):
                ps = psum.tile([128, 512], FP32, tag="ps")
                for db in range(NDB):
                    nc.tensor.matmul(
                        ps[:, : s1 - s0],
                        lhsT=wk_sb[:, db * E + eb * 128: db * E + (eb + 1) * 128],
                        rhs=ktin_sb[:, db * SK + s0: db * SK + s1],
                        start=(db == 0),
                        stop=(db == NDB - 1),
                    )
                nc.vector.tensor_copy(
                    kt_sb[:, eb * SK + s0: eb * SK + s1], ps[:, : s1 - s0]
                )

        # ---- v projection chain: one (kb, eh) psum chain ----
        def v_chain(kb, eh):
            ps = psum.tile([128, 512], FP32, tag="ps")
            for db in range(NDB):
                nc.tensor.matmul(
                    ps[:],
                    lhsT=vtin_sb[:, db * SK + kb * 128: db * SK + (kb + 1) * 128],
                    rhs=wv_sb[:, db * E + eh * 512: db * E + (eh + 1) * 512],
                    start=(db == 0),
                    stop=(db == NDB - 1),
                )
            dst = v3[:, kb, eh * 8:(eh + 1) * 8, 0:HD]
            src = ps[:].rearrange("p (h c) -> p h c", c=HD)
            nc.scalar.copy(dst, src)

        # ---- attention pieces ----
        scale = 1.0 / np.sqrt(HD)
        expp_tiles = {}  # (h, kb) -> (sbuf tile, col base)

        def score_group(h, kbs, mskoff):
            """Score matmuls for 1-2 key blocks sharing one [128,384] PSUM
            tile, one exp, one strided double-block mask multiply."""
            hp = (h % 2) * HD
            he = h // 2
            pscr = pscore.tile([128, 384], FP32, tag="scr")
            col = 0
            for kb in kbs:
                w = KB_QN[kb] * 128
                q0 = KB_Q0[kb] * 128
                nc.tensor.matmul(
                    pscr[:, col:col + w],
                    lhsT=kt_sb[hp:hp + HD,
                               he * SK + kb * 128: he * SK + (kb + 1) * 128],
                    rhs=qt_sb[hp:hp + HD, he * SL + q0: he * SL + q0 + w],
                    start=True,
                    stop=True,
                )
                col += w
            expp = ep.tile([128, 384], BF16, tag=f"expp{h}",
                           name=f"expp_{h}_{kbs[0]}")
            nc.scalar.activation(
                expp[:], pscr[:],
                mybir.ActivationFunctionType.Exp, scale=scale,
            )
            # masked 128-col blocks are always local blocks {0, 2}; SBUF-only
            # op, so it runs on the otherwise-idle GpSimd engine
            ev = expp[:].rearrange("p (b c) -> p b c", b=3)
            dst = ev[:, 0::2]
            src = msk_sb[:, mskoff:mskoff + 256].rearrange(
                "p (b c) -> p b c", b=2)
            nc.gpsimd.tensor_mul(dst, dst, src)
            col = 0
            for kb in kbs:
                expp_tiles[(h, kb)] = (expp, col)
                col += KB_QN[kb] * 128

        def pv_norm(h, qb, ao):
            """PV + normalize for one (head, q-block) into ao tile."""
            ppv = ppv_pool.tile([128, VROW], FP32, tag="pv")
            for r in range(3):
                kb = qb + r
                idx = qb - KB_Q0[kb]
                tile_, base = expp_tiles[(h, kb)]
                off = base + idx * 128
                nc.tensor.matmul(
                    ppv[:],
                    lhsT=tile_[:, off:off + 128],
                    rhs=v_sb[:, (kb * H + h) * VROW:(kb * H + h + 1) * VROW],
                    start=(r == 0),
                    stop=(r == 2),
                )
            rd = rp.tile([128, 1], FP32, tag="rd")
            nc.vector.reciprocal(rd[:], ppv[:, HD:VROW])
            nc.vector.tensor_scalar(
                ao[:, h * HD:(h + 1) * HD],
                ppv[:, 0:HD],
                rd[:],
                None,
                op0=mybir.AluOpType.mult,
            )

        # ---- prologue attention: kb0+kb1 pair tiles + V kb0..2 ----
        for h in range(H):
            score_group(h, (0, 1), 0)
            if h == 3:
                v_chain(0, 0)
            if h == 7:
                v_chain(0, 1)
            if h == 11:
                v_chain(1, 0)
            if h == 14:
                v_chain(1, 1)
        v_chain(2, 0)
        v_chain(2, 1)

        # score groups computed per qb iteration (kb4+kb5 paired at qb2)
        KBN_GROUPS = {0: ((2,), 256), 1: ((3,), 256), 2: ((4, 5), 512)}

        # ---- rolling qb loop ----
        for qb in range(NQB):
            ao = aop.tile([128, E], BF16, tag="ao")
            # one aoT tile PER QUARTER so out-proj matmuls only depend on
            # the XBAR transpose that actually produced their e-blocks
            aot_q = []
            for q4 in range(4):
                t_ = aotp.tile([128, 256], BF16, tag="aot",
                               name=f"aot_{qb}_{q4}")
                aot_q.append(t_)
            grp = KBN_GROUPS.get(qb)

            def opj_mm(ps, eb, dh):
                src = aot_q[eb // 2]
                c0 = (eb % 2) * 128
                nc.tensor.matmul(
                    ps[:],
                    lhsT=src[:, c0:c0 + 128],
                    rhs=wo_sb[:, eb * D + dh * 512: eb * D + (dh + 1) * 512],
                    start=(eb == 0),
                    stop=(eb == NEB - 1),
                )

            def xpose_quarter(q4):
                sync.dma_start_transpose(
                    aot_q[q4][:].rearrange("p (b q) -> p b q", q=128),
                    ao[:, q4 * 256:(q4 + 1) * 256],
                )

            prev = None
            for h in range(H):
                if grp is not None:
                    score_group(h, grp[0], grp[1])
                if qb < NQB - 1 and h in (4, 10):
                    v_chain(qb + 3, 0 if h == 4 else 1)
                if prev is not None:
                    pv_norm(prev, qb, ao)
                    if prev in (3, 7, 11):  # transpose finished quarter
                        xpose_quarter((prev - 3) // 4)
                prev = h
            pv_norm(prev, qb, ao)
            xpose_quarter(3)
            o_t = op.tile([128, D], FP32, tag="o")
            for dh in range(2):
                ps = psum.tile([128, 512], FP32, tag="ps")
                for eb in range(NEB):
                    opj_mm(ps, eb, dh)
                dst = o_t[:, dh * 512:(dh + 1) * 512]
                if dh == 0:
                    nc.vector.tensor_copy(dst, ps[:])
                else:
                    nc.scalar.copy(dst, ps[:])
                sync.dma_start(
                    out_d[qb * 128:(qb + 1) * 128, dh * 512:(dh + 1) * 512],
                    dst,
                )

        for p in reversed(pools):
            p.__exit__(None, None, None)

    nc.compile()
    return nc


def _host_masks():
    bf = ml_dtypes.bfloat16
    kt = np.arange(128)[:, None]
    qi = np.arange(128)[None, :]
    tri0 = (qi <= kt).astype(bf)          # r=0 keep
    tri2 = (kt <= qi).astype(bf)          # r=2 keep
    zeros = np.zeros((128, 128), bf)

    masks = []
    for c in range(NCORES):
        m = np.empty((128, 768), bf)
        m[:, 0:128] = zeros if c == 0 else tri0           # mk0
        m[:, 128:256] = tri0                              # m0 (kb1 pair)
        m[:, 256:384] = tri2                              # m2 (kb2/kb3)
        m[:, 384:512] = tri0                              # m0 (kb2/kb3)
        m[:, 512:640] = tri2                              # m2 (kb4 pair)
        m[:, 640:768] = zeros if c == NCORES - 1 else tri2  # mk5
        masks.append(m)
    return masks


def _host_inputs(query, key, value, Wq, Wk, Wv, Wo):
    bf = ml_dtypes.bfloat16
    q2 = np.ascontiguousarray(query.reshape(S, D))
    k2 = np.asarray(key).reshape(S, D)
    v2 = np.asarray(value).reshape(S, D)
    kpad = np.zeros((S + 2 * WIN, D), np.float32)
    kpad[WIN:WIN + S] = k2
    vpad = np.zeros((S + 2 * WIN, D), np.float32)
    vpad[WIN:WIN + S] = v2

    wq = np.ascontiguousarray(Wq.astype(bf))
    wk = np.ascontiguousarray(Wk.astype(bf))
    wv = np.ascontiguousarray(Wv.astype(bf))
    wo = np.ascontiguousarray(Wo.astype(bf))
    masks = _host_masks()

    in_maps = []
    for c in range(NCORES):
        s0 = c * SL
        qt = np.ascontiguousarray(q2[s0:s0 + SL].T.astype(bf))
        ktc = np.ascontiguousarray(kpad[s0:s0 + SK].T.astype(bf))
        vtc = np.ascontiguousarray(vpad[s0:s0 + SK].T.astype(bf))
        in_maps.append({
            "qt": qt, "kt": ktc, "vt": vtc,
            "wq": wq, "wk": wk, "wv": wv, "wo": wo,
            "msk": masks[c],
        })
    return in_maps


def kernel(query, key, value, Wq, Wk, Wv, Wo):
    global LAST_RESULT
    if "nc" not in _CACHE:
        _CACHE["nc"] = _build_nc()
    nc = _CACHE["nc"]
    in_maps = _host_inputs(
        np.asarray(query), np.asarray(key), np.asarray(value),
        np.asarray(Wq), np.asarray(Wk), np.asarray(Wv), np.asarray(Wo),
    )
    trace = os.environ.get("KERNEL_TRACE", "0") == "1"
    try:
        res = run_bass_kernel_spmd(
            nc, in_maps, core_ids=list(range(NCORES)), trace=trace
        )
    except ModuleNotFoundError:
        res = run_bass_kernel_spmd(
            nc, in_maps, core_ids=list(range(NCORES)), trace=False
        )
    LAST_RESULT = res
    out = np.concatenate([res.results[c]["out"] for c in range(NCORES)], axis=0)
    return out.reshape(1, S, D).astype(np.float32)

